# revision 16
# baseline (speedup 1.0000x reference)
"""AdaptiveGraphLayer Trainium2 kernel (8 NeuronCores, data-parallel over B).

Host precomputes the (x-independent) masked-softmax adjacency, the per-batch
gate (tiny MLP on the temporal-mean context), and algebraically fused weights:

    out = g*(A@x)@Wc1^T + ((g*(A@x)@Wmul^T + b_mul) * x) @ Wo2^T + bc + x
    Wc1 = Wout[:, :D] @ Wadd,  bc = b_out + Wout[:, :D] @ b_add
    A   = diag(gate_b) @ softmax(mask(emb1@emb2^T))         (per batch b)
    residual + b_mul term folded into R = (Wo2 * b_mul[None, :])^T

Device emits s^T = (out - bc)^T in fp16; the host adds x + bc and applies
exact LayerNorm in f32.

Device dataflow per 4-timestep block (fp8 e4m3 DoubleRow for the N x N
aggregation, fp16 elsewhere, f32 PSUM accumulation), software-pipelined so
TensorE/PoolE/VectorE/ScalarE stages of consecutive blocks overlap:
  aggrT[d,n] = x[t](fp8, [p,kt,d]).T @ A^T(fp8, [p,kt,n])  one DoubleRow
               matmul per timestep, K=256                   (TensorE)
  copy aggrT -> SBUF fp16                                   (PoolE/gpsimd)
  m1T [o,tn] = Wmul^T.T @ aggrT   2 x 512-col matmuls       (TensorE)
  mulT[d,tn] = m1T * xT(fp8)                                (VectorE)
  yT  [o,tn] = Wc1T.T @ aggrT + Wo2T.T @ mulT (+ RresT.T @ xT if b_mul!=0)
               accumulating 512-col matmuls, weights stationary (TensorE)
  copy yT -> SBUF fp16                                      (ScalarE)
"""

import numpy as np
import ml_dtypes

BF16 = ml_dtypes.bfloat16
F8 = ml_dtypes.float8_e4m3   # trn2 float8e4
F16 = np.float16

B, T, N, D = 8, 64, 256, 128
P = 128          # partitions
G = N // P       # k-tiles per timestep (2)
TB = 4           # timesteps per PSUM block
NBLK = T // TB
THRESH = 0.01
NCORES = 8

# x DMA segmentation, in blocks (first tiny so compute starts early)
SEGS = [1, 1, 2, 4, 4, 4]

_CACHE = {}


def _build(bres_nonzero: bool):
    from contextlib import ExitStack

    import concourse.tile as tile
    import concourse.mybir as mybir
    from concourse import bacc

    dt = mybir.dt
    Alu = mybir.AluOpType
    DR = mybir.MatmulPerfMode.DoubleRow

    nc = bacc.Bacc("TRN2", target_bir_lowering=False, debug=False,
                   num_devices=NCORES)

    adjt = nc.declare_dram_parameter("adjt", [P, G, N], dt.float8e4, False)
    wc1t = nc.declare_dram_parameter("wc1t", [P, D], dt.float16, False)
    wmult = nc.declare_dram_parameter("wmult", [P, D], dt.float16, False)
    wo2t = nc.declare_dram_parameter("wo2t", [P, D], dt.float16, False)
    if bres_nonzero:
        rres = nc.declare_dram_parameter("rres", [P, D], dt.float16, False)
    x2 = nc.declare_dram_parameter("x2", [P, T, G, D], dt.float8e4, False)
    x2t = nc.declare_dram_parameter("x2t", [P, T, N], dt.float8e4, False)
    out = nc.declare_dram_parameter("out", [P, T, N], dt.float16, True)

    with tile.TileContext(nc) as tc, ExitStack() as ctx:
        consts = ctx.enter_context(tc.tile_pool(name="consts", bufs=1))
        xpool = ctx.enter_context(tc.tile_pool(name="x", bufs=1))
        work = ctx.enter_context(tc.tile_pool(name="work", bufs=5))
        ypool = ctx.enter_context(tc.tile_pool(name="y", bufs=4))
        # 8 PSUM banks: pp 3 x [P,1024] (6 banks, 3 blocks in flight through
        # the a->cast->m1->mul chain) + py 2 x [P,512] (2 banks).
        pp = ctx.enter_context(tc.tile_pool(name="pp", bufs=3, space="PSUM"))
        py = ctx.enter_context(tc.tile_pool(name="py", bufs=2, space="PSUM"))

        adjt_sb = consts.tile([P, G, N], dt.float8e4, tag="adjt")
        wc1t_sb = consts.tile([P, D], dt.float16, tag="wc1t")
        wmult_sb = consts.tile([P, D], dt.float16, tag="wmult")
        wo2t_sb = consts.tile([P, D], dt.float16, tag="wo2t")
        rres_sb = (consts.tile([P, D], dt.float16, tag="rres", name="rres_sb")
                   if bres_nonzero else None)
        xn = [xpool.tile([P, nb * TB, G, D], dt.float8e4, tag=f"xn{q}",
                         name=f"xn{q}") for q, nb in enumerate(SEGS)]
        xt = [xpool.tile([P, nb * TB, N], dt.float8e4, tag=f"xt{q}",
                         name=f"xt{q}") for q, nb in enumerate(SEGS)]

        seg_of = {}
        b0 = 0
        for q, nb in enumerate(SEGS):
            for b in range(b0, b0 + nb):
                seg_of[b] = (q, b - b0)
            b0 += nb

        # Startup-latency-critical transfers first (adjacency + block0 x),
        # remaining segments spread across the other engine queues so no
        # single sequencer serializes the issue stream.
        nc.sync.dma_start(out=adjt_sb[:], in_=adjt[:])
        nc.sync.dma_start(out=xn[0][:], in_=x2[:, 0:SEGS[0] * TB, :, :])
        nc.sync.dma_start(out=wc1t_sb[:], in_=wc1t[:])
        nc.sync.dma_start(out=wmult_sb[:], in_=wmult[:])
        nc.sync.dma_start(out=wo2t_sb[:], in_=wo2t[:])
        if bres_nonzero:
            nc.sync.dma_start(out=rres_sb[:], in_=rres[:])
        # x-feature segments first (stage_a is the pipeline head) on sync,
        # then the transposed copies (first needed ~2 blocks later) on
        # gpsimd.  ScalarE issues nothing: it is the PSUM-exit bottleneck.
        t0 = SEGS[0] * TB
        for q in range(1, len(SEGS)):
            nt = SEGS[q] * TB
            nc.sync.dma_start(out=xn[q][:], in_=x2[:, t0:t0 + nt, :, :])
            t0 += nt
        nc.gpsimd.dma_start(out=xt[0][:], in_=x2t[:, 0:SEGS[0] * TB, :])
        t0 = SEGS[0] * TB
        for q in range(1, len(SEGS)):
            nt = SEGS[q] * TB
            nc.gpsimd.dma_start(out=xt[q][:], in_=x2t[:, t0:t0 + nt, :])
            t0 += nt

        def xn_sl(b, ti):
            q, lb = seg_of[b]
            return xn[q][:, lb * TB + ti, :, :]

        def xt_sl(b):
            q, lb = seg_of[b]
            return xt[q][:, lb * TB:(lb + 1) * TB, :]

        # HAM warm-up: dummy matmuls on a memset tile (no DMA dependency, so
        # the PE clock ramps while the first x tiles are still in flight).
        wz = consts.tile([P, 256], dt.float8e4, tag="wz", name="wz")
        nc.vector.memset(wz[:], 0)
        warm = pp.tile([P, 512], dt.float32, tag="pp", name="warm")
        for w in range(20):
            nc.tensor.matmul(warm[:, :256], wz[:, 0:128],
                             wz[:], start=True, stop=True)

        agg_tiles = {}
        mul_tiles = {}
        pp_tiles = {}

        def stage_a(b):
            # aggrT[d, n] for TB timesteps: one fp8 DoubleRow matmul each
            # (K = 256 source nodes as 2 k-tiles) -> PSUM f32 -> SBUF fp16.
            pa_t = pp.tile([P, TB * N], dt.float32, tag="pp", name="pa_t")
            pp_tiles[b] = pa_t
            for ti in range(TB):
                nc.tensor.matmul(
                    pa_t[:, ti * N:(ti + 1) * N],
                    xn_sl(b, ti),
                    adjt_sb[:],
                    start=True, stop=True, perf_mode=DR,
                )
            # PSUM exits are the structural bottleneck: ScalarE reads PSUM
            # faster (172+FD @1.2GHz) than VectorE (120+FD @0.96GHz), and
            # VectorE also owns the tensor_tensor, so give ScalarE the
            # bigger share of the cast.
            agg_sb = work.tile([P, TB * N], dt.float16, tag="agg",
                               name="agg_sb")
            nc.scalar.copy(out=agg_sb[:, 0:608], in_=pa_t[:, 0:608])
            nc.vector.tensor_copy(out=agg_sb[:, 608:1024], in_=pa_t[:, 608:1024])
            agg_tiles[b] = agg_sb

        def stage_m(b):
            # m1T = Wmul @ aggrT ; mulT = m1T * xT -> SBUF fp16
            agg_sb = agg_tiles[b]
            pm_t = pp_tiles.pop(b)
            for c in range(2):
                nc.tensor.matmul(
                    pm_t[:, c * 512:(c + 1) * 512],
                    wmult_sb[:],
                    agg_sb[:, c * 512:(c + 1) * 512],
                    start=True, stop=True,
                )
            mul_sb = work.tile([P, TB * N], dt.float16, tag="mul",
                               name="mul_sb")
            nc.vector.tensor_tensor(
                out=mul_sb[:].rearrange("p (t n) -> p t n", t=TB),
                in0=pm_t[:].rearrange("p (t n) -> p t n", t=TB),
                in1=xt_sl(b),
                op=Alu.mult,
            )
            mul_tiles[b] = mul_sb

        def stage_s(b):
            # yT[o, tn] = Wc1T.T @ aggrT + Wo2T.T @ mulT (+ RresT.T @ xT);
            # weights stationary, activations moving.  Residual + LayerNorm
            # are applied on the host.
            agg_sb = agg_tiles.pop(b)
            mul_sb = mul_tiles.pop(b)
            xts = xt_sl(b).rearrange("p t n -> p (t n)") if bres_nonzero \
                else None
            y_sb = ypool.tile([P, TB, N], dt.float16, tag="ysb", name="y_sb")
            for c in range(2):
                py_t = py.tile([P, 512], dt.float32, tag="py", name="py_t")
                nc.tensor.matmul(py_t[:], wc1t_sb[:],
                                 agg_sb[:, c * 512:(c + 1) * 512],
                                 start=True, stop=False)
                nc.tensor.matmul(py_t[:], wo2t_sb[:],
                                 mul_sb[:, c * 512:(c + 1) * 512],
                                 start=False, stop=not bres_nonzero)
                if bres_nonzero:
                    nc.tensor.matmul(py_t[:], rres_sb[:],
                                     xts[:, c * 512:(c + 1) * 512],
                                     start=False, stop=True)
                nc.scalar.copy(
                    out=y_sb[:, 2 * c:2 * c + 2, :],
                    in_=py_t[:].rearrange("p (t n) -> p t n", t=2),
                )
            t0 = b * TB
            nc.gpsimd.dma_start(out=out[:, t0:t0 + TB, :], in_=y_sb[:])

        # 3-deep software pipeline: A(b) || M(b-1) || S(b-2).  stage_a is
        # issued first each round: its inputs are DMA-only, so the tensor
        # queue always has ready work while the casts of b-1 drain.
        for i in range(NBLK + 2):
            if i < NBLK:
                stage_a(i)
            if 1 <= i < NBLK + 1:
                stage_m(i - 1)
            if i >= 2:
                stage_s(i - 2)

    nc.compile()
    return nc


def _softmax(x, axis=-1):
    m = np.max(x, axis=axis, keepdims=True)
    e = np.exp(x - m)
    return e / np.sum(e, axis=axis, keepdims=True)


TRACE = False


def _ensure_profile_hook():
    """Register the NTFF profile hook if the image's antenv lacks it."""
    import sys
    import types
    try:
        from antenv import axon_hooks  # noqa: F401
        return
    except ImportError:
        pass
    try:
        from trn_agent_boot.trn_boot import _ntff_profile_via_ctypes
        hook = _ntff_profile_via_ctypes("/opt/axon/libaxon_pjrt.so")
    except Exception:
        hook = None
    mod = types.ModuleType("antenv.axon_hooks")
    mod.get_axon_ntff_profile_hook = lambda: hook
    mod.set_axon_ntff_profile_hook = lambda h: None
    sys.modules["antenv.axon_hooks"] = mod


LDW_OPT = False


def _patch_ldw_opt():
    import concourse.bass_utils as bu
    if getattr(bu, "_ldw_patched", False):
        return
    orig = bu.run_command

    def patched(argv, **kw):
        argv = ["--enable-ldw-opt=true" if a == "--enable-ldw-opt=false" else a
                for a in argv]
        return orig(argv, **kw)

    bu.run_command = patched
    bu._ldw_patched = True


def kernel(x, emb1, emb2, W_add, b_add, W_mul, b_mul, Wa1, ba1, Wa2, ba2,
           W_out, b_out, gamma, beta):
    import concourse.bass_utils as bass_utils
    from concourse.bass_utils import run_bass_kernel_spmd
    if LDW_OPT:
        _patch_ldw_opt()
    if TRACE:
        _ensure_profile_hook()
        bass_utils.upload_artifacts = lambda tmpdir: tmpdir

    x = np.asarray(x, np.float32)
    emb1 = np.asarray(emb1, np.float32)
    emb2 = np.asarray(emb2, np.float32)
    W_add = np.asarray(W_add, np.float32)
    b_add = np.asarray(b_add, np.float32)
    W_mul = np.asarray(W_mul, np.float32)
    b_mul = np.asarray(b_mul, np.float32)
    Wa1 = np.asarray(Wa1, np.float32)
    ba1 = np.asarray(ba1, np.float32)
    Wa2 = np.asarray(Wa2, np.float32)
    ba2 = np.asarray(ba2, np.float32)
    W_out = np.asarray(W_out, np.float32)
    b_out = np.asarray(b_out, np.float32)
    gamma = np.asarray(gamma, np.float32)
    beta = np.asarray(beta, np.float32)

    # ---- host: shared adjacency + per-batch gate ----
    raw = emb1 @ emb2.T
    masked = np.where(raw > THRESH, raw, np.float32(-1e9))
    adj = _softmax(masked, -1)                        # [N, N]
    ctx_m = x.mean(axis=1)                            # [B, N, D]
    h = np.maximum(ctx_m @ Wa1.T + ba1, 0.0)
    gate = 1.0 / (1.0 + np.exp(-(h @ Wa2.T + ba2)))   # [B, N, 1]
    gate = gate[..., 0]                               # [B, N]

    W_out1 = W_out[:, :D]
    W_out2 = W_out[:, D:]
    Wc1 = W_out1 @ W_add                              # [o, d]
    bc = b_out + W_out1 @ b_add
    bres_nonzero = bool(np.any(b_mul != 0.0))

    key = bres_nonzero
    if key not in _CACHE:
        _CACHE[key] = _build(bres_nonzero)
    nc = _CACHE[key]

    wc1t_np = np.ascontiguousarray(Wc1.T).astype(F16)
    wmult_np = np.ascontiguousarray(W_mul.T).astype(F16)
    wo2t_np = np.ascontiguousarray(W_out2.T).astype(F16)
    rres_np = np.ascontiguousarray((W_out2 * b_mul[None, :]).T).astype(F16)

    in_maps = []
    for b in range(NCORES):
        A_b = adj * gate[b][:, None]                  # [n, n']
        adjt_np = np.ascontiguousarray(
            A_b.T.reshape(G, P, N).transpose(1, 0, 2)).astype(F8)
        xb = x[b]                                     # [T, N, D]
        x2_np = np.ascontiguousarray(
            xb.reshape(T, G, P, D).transpose(2, 0, 1, 3)).astype(F8)
        x2t_np = np.ascontiguousarray(
            xb.transpose(2, 0, 1)).astype(F8)         # [D, T, N]
        m = {
            "adjt": adjt_np, "wc1t": wc1t_np, "wmult": wmult_np,
            "wo2t": wo2t_np, "x2": x2_np, "x2t": x2t_np,
        }
        if bres_nonzero:
            m["rres"] = rres_np
        in_maps.append(m)

    res = run_bass_kernel_spmd(nc, in_maps, core_ids=list(range(NCORES)),
                               trace=TRACE)
    import kernel as _self
    _self.LAST_RESULT = res

    outs = np.empty((B, T, N, D), np.float32)
    for b in range(NCORES):
        s = np.asarray(res.results[b]["out"]).astype(np.float32)
        # s: [D, T, N] = yT matmul update; y = x + s^T + bc, then LayerNorm.
        y = s.transpose(1, 2, 0) + x[b] + bc
        mean = y.mean(-1, keepdims=True)
        var = y.var(-1, keepdims=True)
        outs[b] = (y - mean) / np.sqrt(var + 1e-5)

    if np.any(gamma != 1.0) or np.any(beta != 0.0):
        outs = outs * gamma + beta
    return outs


LAST_RESULT = None


# revision 18
# speedup vs baseline: 1.0186x; 1.0186x over previous
"""AdaptiveGraphLayer Trainium2 kernel (8 NeuronCores, data-parallel over B).

Host precomputes the (x-independent) masked-softmax adjacency, the per-batch
gate (tiny MLP on the temporal-mean context), and algebraically fused weights:

    out = g*(A@x)@Wc1^T + ((g*(A@x)@Wmul^T + b_mul) * x) @ Wo2^T + bc + x
    Wc1 = Wout[:, :D] @ Wadd,  bc = b_out + Wout[:, :D] @ b_add
    A   = diag(gate_b) @ softmax(mask(emb1@emb2^T))         (per batch b)
    residual + b_mul term folded into R = (Wo2 * b_mul[None, :])^T

Device emits s^T = (out - bc)^T in fp16; the host adds x + bc and applies
exact LayerNorm in f32.

Device dataflow per 4-timestep block (fp8 e4m3 DoubleRow for the N x N
aggregation, fp16 elsewhere, f32 PSUM accumulation), software-pipelined so
TensorE/PoolE/VectorE/ScalarE stages of consecutive blocks overlap:
  aggrT[d,n] = x[t](fp8, [p,kt,d]).T @ A^T(fp8, [p,kt,n])  one DoubleRow
               matmul per timestep, K=256                   (TensorE)
  copy aggrT -> SBUF fp16                                   (PoolE/gpsimd)
  m1T [o,tn] = Wmul^T.T @ aggrT   2 x 512-col matmuls       (TensorE)
  mulT[d,tn] = m1T * xT(fp8)                                (VectorE)
  yT  [o,tn] = Wc1T.T @ aggrT + Wo2T.T @ mulT (+ RresT.T @ xT if b_mul!=0)
               accumulating 512-col matmuls, weights stationary (TensorE)
  copy yT -> SBUF fp16                                      (ScalarE)
"""

import numpy as np
import ml_dtypes

BF16 = ml_dtypes.bfloat16
F8 = ml_dtypes.float8_e4m3   # trn2 float8e4
F16 = np.float16

B, T, N, D = 8, 64, 256, 128
P = 128          # partitions
G = N // P       # k-tiles per timestep (2)
TB = 4           # timesteps per PSUM block
NBLK = T // TB
THRESH = 0.01
NCORES = 8

# x DMA segmentation, in blocks (first tiny so compute starts early)
SEGS = [1, 1, 2, 4, 4, 4]

_CACHE = {}


def _build(bres_nonzero: bool):
    from contextlib import ExitStack

    import concourse.tile as tile
    import concourse.mybir as mybir
    from concourse import bacc

    dt = mybir.dt
    Alu = mybir.AluOpType
    DR = mybir.MatmulPerfMode.DoubleRowSwInterleave

    nc = bacc.Bacc("TRN2", target_bir_lowering=False, debug=False,
                   num_devices=NCORES)

    adjt = nc.declare_dram_parameter("adjt", [P, G, N], dt.float8e4, False)
    wc1t = nc.declare_dram_parameter("wc1t", [P, D], dt.float16, False)
    wmult = nc.declare_dram_parameter("wmult", [P, D], dt.float16, False)
    wo2t = nc.declare_dram_parameter("wo2t", [P, D], dt.float16, False)
    if bres_nonzero:
        rres = nc.declare_dram_parameter("rres", [P, D], dt.float16, False)
    x2 = nc.declare_dram_parameter("x2", [P, T, G, D], dt.float8e4, False)
    x2t = nc.declare_dram_parameter("x2t", [P, T, N], dt.float8e4, False)
    out = nc.declare_dram_parameter("out", [P, T, N], dt.float16, True)

    with tile.TileContext(nc) as tc, ExitStack() as ctx:
        consts = ctx.enter_context(tc.tile_pool(name="consts", bufs=1))
        xpool = ctx.enter_context(tc.tile_pool(name="x", bufs=1))
        work = ctx.enter_context(tc.tile_pool(name="work", bufs=5))
        ypool = ctx.enter_context(tc.tile_pool(name="y", bufs=4))
        # 8 PSUM banks: pp 3 x [P,1024] (6 banks, 3 blocks in flight through
        # the a->cast->m1->mul chain) + py 2 x [P,512] (2 banks).
        pp = ctx.enter_context(tc.tile_pool(name="pp", bufs=3, space="PSUM"))
        py = ctx.enter_context(tc.tile_pool(name="py", bufs=2, space="PSUM"))

        adjt_sb = consts.tile([P, G, N], dt.float8e4, tag="adjt")
        wc1t_sb = consts.tile([P, D], dt.float16, tag="wc1t")
        wmult_sb = consts.tile([P, D], dt.float16, tag="wmult")
        wo2t_sb = consts.tile([P, D], dt.float16, tag="wo2t")
        rres_sb = (consts.tile([P, D], dt.float16, tag="rres", name="rres_sb")
                   if bres_nonzero else None)
        xn = [xpool.tile([P, nb * TB, G, D], dt.float8e4, tag=f"xn{q}",
                         name=f"xn{q}") for q, nb in enumerate(SEGS)]
        xt = [xpool.tile([P, nb * TB, N], dt.float8e4, tag=f"xt{q}",
                         name=f"xt{q}") for q, nb in enumerate(SEGS)]

        seg_of = {}
        b0 = 0
        for q, nb in enumerate(SEGS):
            for b in range(b0, b0 + nb):
                seg_of[b] = (q, b - b0)
            b0 += nb

        # Startup-latency-critical transfers first (adjacency + block0 x),
        # remaining segments spread across the other engine queues so no
        # single sequencer serializes the issue stream.
        nc.sync.dma_start(out=adjt_sb[:], in_=adjt[:])
        nc.sync.dma_start(out=xn[0][:], in_=x2[:, 0:SEGS[0] * TB, :, :])
        nc.sync.dma_start(out=wc1t_sb[:], in_=wc1t[:])
        nc.sync.dma_start(out=wmult_sb[:], in_=wmult[:])
        nc.sync.dma_start(out=wo2t_sb[:], in_=wo2t[:])
        if bres_nonzero:
            nc.sync.dma_start(out=rres_sb[:], in_=rres[:])
        # x-feature segments first (stage_a is the pipeline head) on sync,
        # then the transposed copies (first needed ~2 blocks later) on
        # gpsimd.  ScalarE issues nothing: it is the PSUM-exit bottleneck.
        t0 = SEGS[0] * TB
        for q in range(1, len(SEGS)):
            nt = SEGS[q] * TB
            nc.sync.dma_start(out=xn[q][:], in_=x2[:, t0:t0 + nt, :, :])
            t0 += nt
        nc.gpsimd.dma_start(out=xt[0][:], in_=x2t[:, 0:SEGS[0] * TB, :])
        t0 = SEGS[0] * TB
        for q in range(1, len(SEGS)):
            nt = SEGS[q] * TB
            nc.gpsimd.dma_start(out=xt[q][:], in_=x2t[:, t0:t0 + nt, :])
            t0 += nt

        def xn_sl(b, ti):
            q, lb = seg_of[b]
            return xn[q][:, lb * TB + ti, :, :]

        def xt_sl(b):
            q, lb = seg_of[b]
            return xt[q][:, lb * TB:(lb + 1) * TB, :]

        # HAM warm-up: dummy matmuls on a memset tile (no DMA dependency, so
        # the PE clock ramps while the first x tiles are still in flight).
        wz = consts.tile([P, 256], dt.float8e4, tag="wz", name="wz")
        nc.vector.memset(wz[:], 0)
        warm = pp.tile([P, 512], dt.float32, tag="pp", name="warm")
        for w in range(20):
            nc.tensor.matmul(warm[:, :256], wz[:, 0:128],
                             wz[:], start=True, stop=True)

        agg_tiles = {}
        mul_tiles = {}
        pp_tiles = {}

        def stage_a(b):
            # aggrT[d, n] for TB timesteps: one fp8 DoubleRow matmul each
            # (K = 256 source nodes as 2 k-tiles) -> PSUM f32 -> SBUF fp16.
            pa_t = pp.tile([P, TB * N], dt.float32, tag="pp", name="pa_t")
            pp_tiles[b] = pa_t
            for ti in range(TB):
                nc.tensor.matmul(
                    pa_t[:, ti * N:(ti + 1) * N],
                    xn_sl(b, ti),
                    adjt_sb[:],
                    start=True, stop=True, perf_mode=DR,
                )
            # PSUM exits are the structural bottleneck: ScalarE reads PSUM
            # faster (172+FD @1.2GHz) than VectorE (120+FD @0.96GHz), and
            # VectorE also owns the tensor_tensor, so give ScalarE the
            # bigger share of the cast.
            agg_sb = work.tile([P, TB * N], dt.float16, tag="agg",
                               name="agg_sb")
            nc.scalar.copy(out=agg_sb[:, 0:608], in_=pa_t[:, 0:608])
            nc.vector.tensor_copy(out=agg_sb[:, 608:1024], in_=pa_t[:, 608:1024])
            agg_tiles[b] = agg_sb

        def stage_m(b):
            # m1T = Wmul @ aggrT ; mulT = m1T * xT -> SBUF fp16
            agg_sb = agg_tiles[b]
            pm_t = pp_tiles.pop(b)
            for c in range(2):
                nc.tensor.matmul(
                    pm_t[:, c * 512:(c + 1) * 512],
                    wmult_sb[:],
                    agg_sb[:, c * 512:(c + 1) * 512],
                    start=True, stop=True,
                )
            mul_sb = work.tile([P, TB * N], dt.float16, tag="mul",
                               name="mul_sb")
            nc.vector.tensor_tensor(
                out=mul_sb[:].rearrange("p (t n) -> p t n", t=TB),
                in0=pm_t[:].rearrange("p (t n) -> p t n", t=TB),
                in1=xt_sl(b),
                op=Alu.mult,
            )
            mul_tiles[b] = mul_sb

        def stage_s(b):
            # yT[o, tn] = Wc1T.T @ aggrT + Wo2T.T @ mulT (+ RresT.T @ xT);
            # weights stationary, activations moving.  Residual + LayerNorm
            # are applied on the host.
            agg_sb = agg_tiles.pop(b)
            mul_sb = mul_tiles.pop(b)
            xts = xt_sl(b).rearrange("p t n -> p (t n)") if bres_nonzero \
                else None
            y_sb = ypool.tile([P, TB, N], dt.float16, tag="ysb", name="y_sb")
            for c in range(2):
                py_t = py.tile([P, 512], dt.float32, tag="py", name="py_t")
                nc.tensor.matmul(py_t[:], wc1t_sb[:],
                                 agg_sb[:, c * 512:(c + 1) * 512],
                                 start=True, stop=False)
                nc.tensor.matmul(py_t[:], wo2t_sb[:],
                                 mul_sb[:, c * 512:(c + 1) * 512],
                                 start=False, stop=not bres_nonzero)
                if bres_nonzero:
                    nc.tensor.matmul(py_t[:], rres_sb[:],
                                     xts[:, c * 512:(c + 1) * 512],
                                     start=False, stop=True)
                nc.scalar.copy(
                    out=y_sb[:, 2 * c:2 * c + 2, :],
                    in_=py_t[:].rearrange("p (t n) -> p t n", t=2),
                )
            t0 = b * TB
            nc.gpsimd.dma_start(out=out[:, t0:t0 + TB, :], in_=y_sb[:])

        # 3-deep software pipeline: A(b) || M(b-1) || S(b-2).  stage_a is
        # issued first each round: its inputs are DMA-only, so the tensor
        # queue always has ready work while the casts of b-1 drain.
        for i in range(NBLK + 2):
            if i < NBLK:
                stage_a(i)
            if 1 <= i < NBLK + 1:
                stage_m(i - 1)
            if i >= 2:
                stage_s(i - 2)

    nc.compile()
    return nc


def _softmax(x, axis=-1):
    m = np.max(x, axis=axis, keepdims=True)
    e = np.exp(x - m)
    return e / np.sum(e, axis=axis, keepdims=True)


TRACE = False


def _ensure_profile_hook():
    """Register the NTFF profile hook if the image's antenv lacks it."""
    import sys
    import types
    try:
        from antenv import axon_hooks  # noqa: F401
        return
    except ImportError:
        pass
    try:
        from trn_agent_boot.trn_boot import _ntff_profile_via_ctypes
        hook = _ntff_profile_via_ctypes("/opt/axon/libaxon_pjrt.so")
    except Exception:
        hook = None
    mod = types.ModuleType("antenv.axon_hooks")
    mod.get_axon_ntff_profile_hook = lambda: hook
    mod.set_axon_ntff_profile_hook = lambda h: None
    sys.modules["antenv.axon_hooks"] = mod


LDW_OPT = False


def _patch_ldw_opt():
    import concourse.bass_utils as bu
    if getattr(bu, "_ldw_patched", False):
        return
    orig = bu.run_command

    def patched(argv, **kw):
        argv = ["--enable-ldw-opt=true" if a == "--enable-ldw-opt=false" else a
                for a in argv]
        return orig(argv, **kw)

    bu.run_command = patched
    bu._ldw_patched = True


def kernel(x, emb1, emb2, W_add, b_add, W_mul, b_mul, Wa1, ba1, Wa2, ba2,
           W_out, b_out, gamma, beta):
    import concourse.bass_utils as bass_utils
    from concourse.bass_utils import run_bass_kernel_spmd
    if LDW_OPT:
        _patch_ldw_opt()
    if TRACE:
        _ensure_profile_hook()
        bass_utils.upload_artifacts = lambda tmpdir: tmpdir

    x = np.asarray(x, np.float32)
    emb1 = np.asarray(emb1, np.float32)
    emb2 = np.asarray(emb2, np.float32)
    W_add = np.asarray(W_add, np.float32)
    b_add = np.asarray(b_add, np.float32)
    W_mul = np.asarray(W_mul, np.float32)
    b_mul = np.asarray(b_mul, np.float32)
    Wa1 = np.asarray(Wa1, np.float32)
    ba1 = np.asarray(ba1, np.float32)
    Wa2 = np.asarray(Wa2, np.float32)
    ba2 = np.asarray(ba2, np.float32)
    W_out = np.asarray(W_out, np.float32)
    b_out = np.asarray(b_out, np.float32)
    gamma = np.asarray(gamma, np.float32)
    beta = np.asarray(beta, np.float32)

    # ---- host: shared adjacency + per-batch gate ----
    raw = emb1 @ emb2.T
    masked = np.where(raw > THRESH, raw, np.float32(-1e9))
    adj = _softmax(masked, -1)                        # [N, N]
    ctx_m = x.mean(axis=1)                            # [B, N, D]
    h = np.maximum(ctx_m @ Wa1.T + ba1, 0.0)
    gate = 1.0 / (1.0 + np.exp(-(h @ Wa2.T + ba2)))   # [B, N, 1]
    gate = gate[..., 0]                               # [B, N]

    W_out1 = W_out[:, :D]
    W_out2 = W_out[:, D:]
    Wc1 = W_out1 @ W_add                              # [o, d]
    bc = b_out + W_out1 @ b_add
    bres_nonzero = bool(np.any(b_mul != 0.0))

    key = bres_nonzero
    if key not in _CACHE:
        _CACHE[key] = _build(bres_nonzero)
    nc = _CACHE[key]

    wc1t_np = np.ascontiguousarray(Wc1.T).astype(F16)
    wmult_np = np.ascontiguousarray(W_mul.T).astype(F16)
    wo2t_np = np.ascontiguousarray(W_out2.T).astype(F16)
    rres_np = np.ascontiguousarray((W_out2 * b_mul[None, :]).T).astype(F16)

    in_maps = []
    for b in range(NCORES):
        A_b = adj * gate[b][:, None]                  # [n, n']
        adjt_np = np.ascontiguousarray(
            A_b.T.reshape(G, P, N).transpose(1, 0, 2)).astype(F8)
        xb = x[b]                                     # [T, N, D]
        # DoubleRowSwInterleave weight layout: the PE reads the stationary
        # flat; position f = 2*(127-d) + kt holds x[t, kt*128+p, d].
        x2_np = np.ascontiguousarray(
            xb.reshape(T, G, P, D).transpose(2, 0, 1, 3)[:, :, :, ::-1]
            .transpose(0, 1, 3, 2)).astype(F8)
        x2t_np = np.ascontiguousarray(
            xb.transpose(2, 0, 1)).astype(F8)         # [D, T, N]
        m = {
            "adjt": adjt_np, "wc1t": wc1t_np, "wmult": wmult_np,
            "wo2t": wo2t_np, "x2": x2_np, "x2t": x2t_np,
        }
        if bres_nonzero:
            m["rres"] = rres_np
        in_maps.append(m)

    res = run_bass_kernel_spmd(nc, in_maps, core_ids=list(range(NCORES)),
                               trace=TRACE)
    import kernel as _self
    _self.LAST_RESULT = res

    outs = np.empty((B, T, N, D), np.float32)
    for b in range(NCORES):
        s = np.asarray(res.results[b]["out"]).astype(np.float32)
        # s: [D, T, N] = yT matmul update; y = x + s^T + bc, then LayerNorm.
        y = s.transpose(1, 2, 0) + x[b] + bc
        mean = y.mean(-1, keepdims=True)
        var = y.var(-1, keepdims=True)
        outs[b] = (y - mean) / np.sqrt(var + 1e-5)

    if np.any(gamma != 1.0) or np.any(beta != 0.0):
        outs = outs * gamma + beta
    return outs


LAST_RESULT = None
